# revision 30
# baseline (speedup 1.0000x reference)
"""AlphaQubit-like recurrent transformer on 8 TRN2 NeuronCores.

Strategy:
- Data-parallel over batch: B=16 -> 2 per core, params replicated, no
  collectives. Host shards inputs / concatenates outputs.
- Feature-major on-device layout: activations [d=128 partitions, tokens free].
- The two batches per core run as two independent dependency chains
  ([d, 120] tiles) so their ops interleave across engines.
- Embedding chunks (one per cycle t) are emitted inside the main loop two
  iterations ahead; the Tile scheduler uses them to fill engine gaps.
- bf16 matmul operands, fp32 PSUM accumulation.
- Single ACT table set (natural_log_exp_and_others): LayerNorm rstd via
  exp(-0.5*ln(var+eps)), softmax via exp, gelu via exp-form tanh approx.
- LN scale/bias, attention scale, and small biases folded into weights on host.
- Readout tail (post-recurrence LN + conv head + residual MLP, <1% of FLOPs)
  computed on host in fp32.
"""

import math
import os
import sys

import numpy as np

sys.path.insert(0, "/opt/trn_rl_repo")

import concourse.bass as bass
import concourse.bacc as bacc
import concourse.tile as tile
from concourse.tile import add_dep_helper
from concourse import mybir
from concourse.bass_utils import run_bass_kernel_spmd

import ml_dtypes

BF16 = ml_dtypes.bfloat16

# model dims
B, T, S, D = 16, 8, 120, 128
L, H, DA, DM, DB = 2, 4, 32, 32, 32
NCORES = 8
B2 = B // NCORES          # 2 batches per core
N = B2 * S                # 240 tokens (both batches)
NE = T * B2 * S           # 1920 tokens in embed phase
GRID = 12
RD, NRB = 48, 16

# gelu (tanh approx) constants, computed via exp:
#   gelu(x) ~= x * sigmoid(2u), u = sqrt(2/pi) * (x + r*x^3)
#   e = exp(-2u) = exp(sg * r * (x^2 + 1/r) * x)
R_G = 0.044715
SG = -2.0 * math.sqrt(2.0 / math.pi)
EXP_SCALE = SG * R_G     # ACT scale for exp input (applied to (x^2+1/r)*x)
INV_RG = 1.0 / R_G

F32 = mybir.dt.float32
BF = mybir.dt.bfloat16
AF = mybir.ActivationFunctionType
ALU = mybir.AluOpType

_CACHE = {}

# wall segment layout (shared by device + host): first segments feed the
# embed phase so its compute can start while the rest still streams in.
WALL_SEGS = [
    ("m4", 4, NE), ("ce", D, NE), ("w4", 4, D), ("wer", D, 2 * 2 * D),
    ("ident", S, S), ("bpt", S, L * B2 * H * S),
    ("wqk", D, L * 2 * D), ("wv", D, L * D), ("wo", D, L * D),
    ("wf1", D, L * 4 * D), ("wf2", D, L * 2 * D), ("wcv", D, L * 3 * D),
    ("hsel", S, 16), ("ind4", 4, D),
]
WALL_COLS = sum(c for _, _, c in WALL_SEGS)
EARLY_COLS = 2 * NE + D + 4 * D  # m4, ce, w4, wer


# --------------------------------------------------------------------------
# device graph
# --------------------------------------------------------------------------

def _patched_act_tables(arch):
    # The stock picker maps Ln->natural_log and Exp->exp_and_others,
    # reloading the ACT table (~1.3us) on every switch. Empty those two
    # sets so both functions resolve to natural_log_exp_and_others
    # (positional set ids must stay intact).
    from concourse.hw_specs import get_activation_tables as real
    tabs = dict(real(arch))
    out = {}
    for k, v in tabs.items():
        if k in ("natural_log", "exp_and_others", "exp_and_friends"):
            out[k] = set()
        else:
            out[k] = v
    return out


def build_graph():
    bacc_mod = sys.modules["concourse.bacc"]
    bacc_mod.get_activation_tables = _patched_act_tables
    nc = bacc.Bacc(None)

    wall = nc.declare_dram_parameter("wall", [D, WALL_COLS], BF, isOutput=False)
    bpp = nc.declare_dram_parameter("bpp", [D, 150], F32, isOutput=False)
    xout = nc.declare_dram_parameter("xout", [D, N], F32, isOutput=True)

    # per-partition bias column indices in bpp
    BO = lambda l: 4 + l            # 4,5
    BF2 = lambda l: 6 + l           # 6,7
    BER2 = lambda r: 8 + r          # 8,9
    BF1 = lambda l, s: 10 + l * 4 + s   # 10..17  (s in 0..3: a0,a1,g0,g1)
    BCV = lambda l: 18 + l          # 18,19
    BER1 = lambda r: 20 + r         # 20,21

    with tile.TileContext(nc) as tc:
        singles = tc.alloc_tile_pool(name="singles", bufs=1)
        work = tc.alloc_tile_pool(name="work", bufs=4)
        xpool = tc.alloc_tile_pool(name="xpool", bufs=4)
        ps = tc.alloc_tile_pool(name="ps", bufs=2, space="PSUM")

        # ---- load constants/weights into SBUF (embed segs first) ----
        s_wall = singles.tile([D, WALL_COLS], BF, tag="wall")
        nc.sync.dma_start(out=s_wall[:, 0:EARLY_COLS], in_=wall[:, 0:EARLY_COLS])
        nc.sync.dma_start(out=s_wall[:, EARLY_COLS:], in_=wall[:, EARLY_COLS:])
        s_bpp = singles.tile([D, 150], F32, tag="bpp")
        nc.sync.dma_start(out=s_bpp, in_=bpp[:, :])

        seg_off = {}
        off = 0
        for nm, rows, cols in WALL_SEGS:
            seg_off[nm] = off
            off += cols

        def seg(nm, rows, cols):
            o = seg_off[nm]
            return s_wall[0:rows, o:o + cols]

        s_m4 = seg("m4", 4, NE)
        s_ce = seg("ce", D, NE)
        s_bpt = seg("bpt", S, L * B2 * H * S)
        s_wqk = seg("wqk", D, L * 2 * D)
        s_wv = seg("wv", D, L * D)
        s_wo = seg("wo", D, L * D)
        s_wf1 = seg("wf1", D, L * 4 * D)
        s_wf2 = seg("wf2", D, L * 2 * D)
        s_wcv = seg("wcv", D, L * 3 * D)
        s_wer = seg("wer", D, 2 * 2 * D)
        s_w4 = seg("w4", 4, D)
        s_id = seg("ident", S, S)

        onesc = singles.tile([D, D], BF)       # 1/128 (stat matmuls)
        nc.vector.memset(onesc, 1.0 / 128.0)
        onescf = singles.tile([D, D], F32)     # 1/128 fp32 (mean of fp32 X)
        nc.vector.memset(onescf, 1.0 / 128.0)
        hsel = seg("hsel", S, 16)
        ind4 = seg("ind4", 4, D)
        ind4f = s_bpp[0:4, 22:22 + D]
        ones1 = singles.tile([D, D], BF)       # 1.0 (denominator/broadcast)
        nc.vector.memset(ones1, 1.0)
        onesf = singles.tile([1, 32], F32)     # 1.0 fp32 (recip broadcast)
        nc.vector.memset(onesf, 1.0)
        eps_t = singles.tile([1, 1], F32)
        nc.vector.memset(eps_t, 1e-5)
        zero_t = singles.tile([D, 1], F32)
        nc.vector.memset(zero_t, 0.0)

        bias_ap = lambda c: s_bpp[:, c:c + 1]

        # ---- helper: layernorm (feature-major) -> xn bf16 [D, n] ----
        # psum arena `slot` [D, 480] (one bank): mean bcast 0:n, var row
        # n:2n, rstd bcast 2n:3n (or a second slot when n > 160).
        def layer_norm(x, n, arena_tag, tp, abufs=2):
            slot = ps.tile([D, 480], F32, tag=arena_tag, bufs=abufs)
            mb = slot[:, 0:n]
            nc.tensor.matmul(mb, onescf, x, start=True, stop=True)
            xc = work.tile([D, n], BF, tag=tp + "ln_xc", bufs=4)
            nc.vector.tensor_sub(xc, x, mb)
            sq = work.tile([D, n], BF, tag=tp + "ln_sq", bufs=4)
            nc.scalar.activation(sq, xc, AF.Square, bias=zero_t, scale=1.0)
            vr = slot[0:1, n:2 * n]
            nc.tensor.matmul(vr, onesc[:, 0:1], sq, start=True, stop=True)
            lnr = work.tile([1, n], F32, tag=tp + "ln_lnr", bufs=4)
            nc.scalar.activation(lnr, vr, AF.Ln, bias=eps_t[0:1, :], scale=1.0)
            rsr = work.tile([1, n], BF, tag=tp + "ln_rsr", bufs=4)
            nc.scalar.activation(rsr, lnr, AF.Exp, bias=zero_t[0:1, :], scale=-0.5)
            if 3 * n <= 480:
                rb = slot[:, 2 * n:3 * n]
            else:
                rb = ps.tile([D, 480], F32, tag=arena_tag, bufs=abufs, name="ln_rbs")[:, 0:n]
            nc.tensor.matmul(rb, ones1[0:1, 0:D], rsr, start=True, stop=True)
            xn = work.tile([D, n], BF, tag=tp + "ln_xn", bufs=4)
            nc.vector.tensor_mul(xn, xc, rb)
            return xn

        # ---- helper: gelu(a) -> gl bf16 [D, n]; a is sbuf bf16 ----
        def gelu(a, n, tag, w_eng=None, gl_eng=None):
            x2 = work.tile([D, n], BF, tag=tag + "_x2")
            nc.scalar.activation(x2, a, AF.Square, bias=zero_t, scale=1.0)
            w = work.tile([D, n], BF, tag=tag + "_w")
            (w_eng or nc.vector).scalar_tensor_tensor(
                w, x2, INV_RG, a, op0=ALU.add, op1=ALU.mult)
            e = work.tile([D, n], F32, tag=tag + "_e")
            nc.scalar.activation(e, w, AF.Exp, bias=zero_t, scale=EXP_SCALE)
            dd = work.tile([D, n], F32, tag=tag + "_dd")
            nc.vector.tensor_scalar_add(dd, e, 1.0)
            rc = work.tile([D, n], F32, tag=tag + "_rc")
            nc.vector.reciprocal_approx_fast(out=rc, in_=dd)
            gl = work.tile([D, n], BF, tag=tag + "_gl")
            (gl_eng or nc.vector).tensor_mul(gl, rc, a)
            return gl

        # ================= embed chunk (one cycle t, both batches) =======
        e_tiles = [None] * T

        def emit_embed_chunk(t):
            sl = slice(t * N, (t + 1) * N)
            h0p = ps.tile([D, 480], F32, tag="E", bufs=2, name="e_h0p")[:, 0:N]
            nc.tensor.matmul(h0p, s_w4, s_m4[:, sl], start=True, stop=True)
            h = work.tile([D, N], F32, tag="emb_h", bufs=3)
            nc.vector.tensor_add(h, h0p, s_ce[:, sl])
            for r in range(2):
                xn = layer_norm(h, N, "E", "e_")
                f1p = ps.tile([D, 480], F32, tag="E", bufs=2, name="e_f1p")[:, 0:N]
                nc.tensor.matmul(f1p, s_wer[:, (r * 2) * D:(r * 2) * D + D], xn,
                                 start=True, stop=True)
                a = work.tile([D, N], BF, tag="emb_a")
                nc.scalar.activation(a, f1p, AF.Identity,
                                     bias=bias_ap(BER1(r)), scale=1.0)
                gl = gelu(a, N, "emb_g", gl_eng=nc.gpsimd)
                f2p = ps.tile([D, 480], F32, tag="E", bufs=2, name="e_f2p")[:, 0:N]
                nc.tensor.matmul(f2p, s_wer[:, (r * 2 + 1) * D:(r * 2 + 1) * D + D],
                                 gl, start=True, stop=True)
                if r == 0:
                    hn = work.tile([D, N], F32, tag="emb_h", bufs=3)
                else:
                    hn = xpool.tile([D, N], F32, tag=f"e{t}", bufs=1)
                nc.vector.scalar_tensor_tensor(
                    hn, f2p, bias_ap(BER2(r)), h, op0=ALU.add, op1=ALU.add)
                h = hn
            e_tiles[t] = h  # fp32 [D, N], pre-scaled by 1/sqrt(2)

        # ================= main loop stages (one batch chain each) =======
        NB = S  # 120 tokens per batch chain

        def attn(X, b, l):
            xn = layer_norm(X, NB, "A", "m_")
            # B-arena bank 1: Q [0:32, (h,i)], K [32:64, (h,i)]
            qkp = ps.tile([D, 480], F32, tag="B", bufs=2, name="qkp")
            for hh in range(H):
                nc.tensor.matmul(
                    qkp[0:DA, hh * NB:(hh + 1) * NB],
                    s_wqk[:, (l * 2) * D + hh * DA:(l * 2) * D + (hh + 1) * DA],
                    xn, start=True, stop=True)
                nc.tensor.matmul(
                    qkp[DA:2 * DA, hh * NB:(hh + 1) * NB],
                    s_wqk[:, (l * 2 + 1) * D + hh * DA:(l * 2 + 1) * D + (hh + 1) * DA],
                    xn, start=True, stop=True,
                    tile_position=(0, DA), skip_group_check=True)
            qst = work.tile([DA, H * NB], BF, tag="qst")
            nc.vector.tensor_copy(qst, qkp[0:DA, :])
            kst = work.tile([DA, H * NB], BF, tag="kst")
            nc.vector.tensor_copy(kst, qkp[DA:2 * DA, :])
            # B-arena bank 2: V [0:120, 0:128], bc 128:248, ot 248:368, zt 368:488
            varena = ps.tile([D, 512], F32, tag="B", bufs=2, name="varena")
            vp = varena[0:NB, 0:D]
            nc.tensor.matmul(vp, xn, s_wv[:, l * D:(l + 1) * D],
                             start=True, stop=True)
            vsb = work.tile([NB, D], BF, tag="vsb")
            nc.vector.tensor_copy(vsb, vp)

            # scores in two head-pair groups so exp(h01) overlaps scores(h23)
            boff = l * B2 * H * S + b * H * S
            ex = work.tile([S, H * S], BF, tag="ex")
            for g in range(2):
                scg = ps.tile([D, 480], F32, tag="C", bufs=2, name="scg")[0:S, 0:2 * S]
                go = 2 * g * S
                nc.tensor.matmul(scg, s_id, s_bpt[:, boff + go:boff + go + 2 * S],
                                 start=True, stop=False)
                for hh in range(2 * g, 2 * g + 2):
                    nc.tensor.matmul(
                        scg[:, (hh - 2 * g) * S:(hh - 2 * g + 1) * S],
                        kst[:, hh * NB:(hh + 1) * NB], qst[:, hh * NB:(hh + 1) * NB],
                        start=False, stop=(hh == 2 * g + 1))
                nc.scalar.activation(ex[:, go:go + 2 * S], scg, AF.Exp,
                                     bias=zero_t[0:S, :], scale=1.0)

            # denominators into [4(h), 120]: row h from head-h block of ex
            dn = ps.tile([D, 480], F32, tag="C", bufs=2, name="dn")[0:4, 0:NB]
            for hh in range(H):
                nc.tensor.matmul(dn, hsel[:, 4 * hh:4 * hh + 4],
                                 ex[:, hh * S:(hh + 1) * S],
                                 start=(hh == 0), stop=(hh == H - 1))
            rr = work.tile([4, NB], F32, tag="rr")
            nc.vector.reciprocal_approx_fast(out=rr, in_=dn)
            nc.tensor.matmul(varena[:, D:D + NB], ind4f, rr,
                             start=True, stop=True)
            for hh in range(H):
                nc.tensor.matmul(varena[hh * DM:(hh + 1) * DM, D + NB:D + 2 * NB],
                                 vsb[:, hh * DM:(hh + 1) * DM],
                                 ex[:, hh * S:(hh + 1) * S],
                                 start=True, stop=True,
                                 tile_position=(0, hh * DM), skip_group_check=True)
            bcs = work.tile([D, NB], BF, tag="bcs")
            nc.scalar.activation(bcs, varena[:, D:D + NB], AF.Copy)
            on = work.tile([D, NB], BF, tag="on")
            nc.vector.tensor_mul(on, varena[:, D + NB:D + 2 * NB], bcs)
            zt = varena[:, D + 2 * NB:D + 3 * NB]
            nc.tensor.matmul(zt, s_wo[:, l * D:(l + 1) * D], on,
                             start=True, stop=True)
            x2t = xpool.tile([D, NB], F32, tag=f"x{b}", bufs=3, name="x2t")
            nc.vector.scalar_tensor_tensor(
                x2t, zt, bias_ap(BO(l)), X, op0=ALU.add, op1=ALU.add)
            return x2t

        def ffn(X, b, l):
            xn2 = layer_norm(X, NB, "A", "m_")
            f1 = ps.tile([D, 480], F32, tag="C", bufs=2, name="f1")
            for s4 in range(4):
                nc.tensor.matmul(
                    f1[:, s4 * NB:(s4 + 1) * NB],
                    s_wf1[:, l * 4 * D + s4 * D: l * 4 * D + (s4 + 1) * D],
                    xn2, start=True, stop=True)
            a_s = work.tile([D, 2 * NB], BF, tag="ffa")
            nc.scalar.activation(a_s, f1[:, 0:2 * NB], AF.Identity,
                                 bias=bias_ap(BF1(l, 0)), scale=1.0)
            gl = gelu(a_s, 2 * NB, "ffg")
            ffo = work.tile([D, 2 * NB], BF, tag="ffo")
            nc.vector.tensor_mul(ffo, f1[:, 2 * NB:4 * NB], gl)
            zf = ps.tile([D, 480], F32, tag="B", bufs=2, name="zf")[:, 0:NB]
            for s2 in range(2):
                nc.tensor.matmul(zf, s_wf2[:, (l * 2 + s2) * D:(l * 2 + s2 + 1) * D],
                                 ffo[:, s2 * NB:(s2 + 1) * NB],
                                 start=(s2 == 0), stop=(s2 == 1))
            x3t = xpool.tile([D, NB], F32, tag=f"x{b}", bufs=3, name="x3t")
            nc.vector.scalar_tensor_tensor(
                x3t, zf, bias_ap(BF2(l)), X, op0=ALU.add, op1=ALU.add)
            return x3t

        def conv(X, b, l):
            x3b = work.tile([D, NB], BF, tag="x3b")
            nc.vector.tensor_copy(x3b, X)
            cv = ps.tile([D, 480], F32, tag="B", bufs=2, name="cv")[:, 0:NB]
            k0 = l * 3 * D
            nc.tensor.matmul(cv, s_wcv[:, k0 + D:k0 + 2 * D], x3b,
                             start=True, stop=False)
            nc.tensor.matmul(cv[:, 1:NB], s_wcv[:, k0:k0 + D],
                             x3b[:, 0:NB - 1], start=False, stop=False)
            nc.tensor.matmul(cv[:, 0:NB - 1], s_wcv[:, k0 + 2 * D:k0 + 3 * D],
                             x3b[:, 1:NB], start=False, stop=True)
            acv = work.tile([D, NB], BF, tag="acv")
            nc.scalar.activation(acv, cv, AF.Identity,
                                 bias=bias_ap(BCV(l)), scale=1.0)
            gl = gelu(acv, NB, "cvg")
            x4t = xpool.tile([D, NB], F32, tag=f"x{b}", bufs=3, name="x4t")
            nc.vector.tensor_add(x4t, gl, X)
            return x4t

        # ================= schedule =================
        emit_embed_chunk(0)
        emit_embed_chunk(1)

        Xb = [None, None]
        for t in range(T):
            e_t = e_tiles[t]
            for b in range(B2):
                esl = e_t[:, b * NB:(b + 1) * NB]
                if t == 0:
                    Xb[b] = esl  # X0 = (0 + E0)/sqrt(2), scaling pre-folded
                else:
                    xnew = xpool.tile([D, NB], F32, tag=f"x{b}", bufs=3)
                    nc.vector.scalar_tensor_tensor(
                        xnew, Xb[b], 1.0 / math.sqrt(2.0), esl,
                        op0=ALU.mult, op1=ALU.add)
                    Xb[b] = xnew
            STAGE = os.environ.get("K_STAGE", "full")
            for l in range(L):
                if STAGE in ("full", "attn", "ffn"):
                    for b in range(B2):
                        Xb[b] = attn(Xb[b], b, l)
                if STAGE in ("full", "ffn"):
                    for b in range(B2):
                        Xb[b] = ffn(Xb[b], b, l)
                if STAGE in ("full",):
                    for b in range(B2):
                        Xb[b] = conv(Xb[b], b, l)
            if t + 2 < T:
                emit_embed_chunk(t + 2)

        # write out final X (f32)
        for b in range(B2):
            nc.sync.dma_start(out=xout[:, b * NB:(b + 1) * NB], in_=Xb[b])

        for p in (ps, xpool, work, singles):
            p.release()

    nc.compile()
    return nc


# --------------------------------------------------------------------------
# host pre/post-processing
# --------------------------------------------------------------------------

def _bf(x):
    return np.asarray(x, dtype=np.float32).astype(BF16)


def prepare_inputs(inp):
    """Build per-core input maps (numpy) from full fp32 inputs."""
    f = {k: np.asarray(v, dtype=np.float32) for k, v in inp.items()
         if k not in ("stab_ids", "cycle_ids")}
    stab_ids = np.asarray(inp["stab_ids"])
    cycle_ids = np.asarray(inp["cycle_ids"])

    scale = 1.0 / math.sqrt(DA)
    isq2 = 1.0 / math.sqrt(2.0)

    # ---- replicated weights ----
    # wqk: ln1-folded, q side also attn-scaled
    wqk = np.zeros((D, L * 2 * D), np.float32)
    bqk = np.zeros((D, 4), np.float32)
    for l in range(L):
        wq = f["Wq"][l].transpose(1, 0, 2).reshape(D, H * DA)   # [d, (h,e)]
        wk = f["Wk"][l].transpose(1, 0, 2).reshape(D, H * DA)
        wq_f = f["ln1_s"][l][:, None] * wq
        wk_f = f["ln1_s"][l][:, None] * wk
        bq_f = (f["bq"][l].reshape(-1) + f["ln1_b"][l] @ wq) * scale
        bk_f = f["bk"][l].reshape(-1) + f["ln1_b"][l] @ wk
        wqk[:, (l * 2) * D:(l * 2) * D + D] = wq_f * scale
        wqk[:, (l * 2 + 1) * D:(l * 2 + 1) * D + D] = wk_f
        bqk[:, l * 2 + 0] = bq_f
        bqk[:, l * 2 + 1] = bk_f

    wv = np.zeros((D, L * D), np.float32)
    wo = np.zeros((D, L * D), np.float32)
    bo_all = np.zeros((D, L), np.float32)
    for l in range(L):
        wv_r = f["Wv"][l].transpose(1, 0, 2).reshape(D, H * DM)
        wv_f = f["ln1_s"][l][:, None] * wv_r
        bv_f = f["bv"][l].reshape(-1) + f["ln1_b"][l] @ wv_r
        wv[:, l * D:(l + 1) * D] = wv_f
        wo[:, l * D:(l + 1) * D] = f["Wo"][l]         # [hm, d]
        bo_all[:, l] = f["bo"][l] + bv_f @ f["Wo"][l]

    wf1 = np.zeros((D, L * 4 * D), np.float32)
    bf1 = np.zeros((D, 8), np.float32)
    for l in range(L):
        w = f["ln2_s"][l][:, None] * f["f1_w"][l]      # [d, 512]
        bias = f["f1_b"][l] + f["ln2_b"][l] @ f["f1_w"][l]
        wf1[:, l * 4 * D:(l + 1) * 4 * D] = w
        for s4 in range(4):
            bf1[:, l * 4 + s4] = bias[s4 * D:(s4 + 1) * D]

    wf2 = np.zeros((D, L * 2 * D), np.float32)
    bf2 = np.zeros((D, L), np.float32)
    for l in range(L):
        for s2 in range(2):
            wf2[:, (l * 2 + s2) * D:(l * 2 + s2 + 1) * D] = \
                f["f2_w"][l][s2 * D:(s2 + 1) * D]
        bf2[:, l] = f["f2_b"][l]

    wcv = np.zeros((D, L * 3 * D), np.float32)
    bcv = np.zeros((D, L), np.float32)
    for l in range(L):
        for k in range(3):
            wcv[:, (l * 3 + k) * D:(l * 3 + k + 1) * D] = f["conv_w"][l][:, :, k].T
        bcv[:, l] = f["conv_b"][l]

    wer = np.zeros((D, 4 * D), np.float32)
    ber1 = np.zeros((D, 2), np.float32)
    ber2 = np.zeros((D, 2), np.float32)
    for r in range(2):
        w1 = f["er_ln_s"][r][:, None] * f["er_fc1_w"][r]
        b1 = f["er_fc1_b"][r] + f["er_ln_b"][r] @ f["er_fc1_w"][r]
        wer[:, (r * 2) * D:(r * 2) * D + D] = w1
        wer[:, (r * 2 + 1) * D:(r * 2 + 1) * D + D] = f["er_fc2_w"][r] * isq2
        ber1[:, r] = b1
        ber2[:, r] = f["er_fc2_b"][r] * isq2

    w4 = np.stack([f["pm_w"], f["pe_w"], f["pl_w"], f["pel_w"]], 0)  # [4,d]

    assert np.abs(bqk).max() == 0.0, "qk biases must be zero (folded path)"
    assert np.abs(bf1).max() == 0.0, "f1 biases must be zero (fused a_s/ffo path)"
    bpp = np.zeros((D, 150), np.float32)
    for g in range(4):
        bpp[g, 22 + 32 * g:22 + 32 * (g + 1)] = 1.0
    bpp[:, 0:4] = bqk
    bpp[:, 4:6] = bo_all
    bpp[:, 6:8] = bf2
    bpp[:, 8:10] = ber2
    bpp[:, 10:18] = bf1
    bpp[:, 18:20] = bcv
    bpp[:, 20:22] = ber1

    # const embedding [d, (t, s)] replicated over b, scaled by 1/sqrt(2)
    pos = f["stab_emb"][stab_ids]                      # [S, d]
    cyc = f["cyc_emb"][cycle_ids]                      # [T, d]
    cbias = f["pm_b"] + f["pe_b"] + f["pl_b"] + f["pel_b"]
    const_ts = (cbias[None, None, :] + pos[None, :, :] + cyc[:, None, :]) * isq2
    # [T, S, d] -> [d, (t,b,s)]
    ce_full = np.repeat(const_ts[:, None, :, :], B2, axis=1)  # [T,B2,S,d]
    ce = ce_full.transpose(3, 0, 1, 2).reshape(D, NE)

    ident = np.eye(S, dtype=np.float32)

    def pack_wall(m4c, bptc):
        hsel = np.zeros((S, 16), np.float32)
        for h in range(H):
            hsel[:, 5 * h] = 1.0
        ind4 = np.zeros((4, D), np.float32)
        for g in range(4):
            ind4[g, 32 * g:32 * (g + 1)] = 1.0
        arrs = {"m4": m4c, "ce": ce, "w4": w4, "wer": wer, "ident": ident,
                "bpt": bptc, "wqk": wqk, "wv": wv, "wo": wo,
                "wf1": wf1, "wf2": wf2, "wcv": wcv,
                "hsel": hsel, "ind4": ind4}
        wallm = np.zeros((D, WALL_COLS), np.float32)
        o = 0
        for nm, r, c in WALL_SEGS:
            wallm[0:r, o:o + c] = arrs[nm]
            o += c
        return _bf(wallm)

    # ---- per-core sharded inputs ----
    # Bp^T: [l, b, h, j(k), i(q)] scaled by 1/sqrt(da)
    bias_in = f["bias"]                                # [B, S, S, DB]
    Wb = f["Wb"]                                       # [L, DB, H]
    bp = np.einsum("bijd,ldh->lbhji", bias_in, Wb) * scale  # [L,B,H,S(j),S(i)]

    in_maps = []
    for c in range(NCORES):
        bsl = slice(c * B2, (c + 1) * B2)
        m4c = np.stack([f["meas"][bsl], f["event"][bsl], f["leak"][bsl],
                        f["event_leak"][bsl]], 0)       # [4, B2, T, S]
        m4c = (m4c.transpose(0, 2, 1, 3).reshape(4, NE)) * isq2  # (t,b,s)
        bptc = bp[:, bsl]                               # [L, B2, H, S, S]
        bptc = bptc.transpose(3, 0, 1, 2, 4).reshape(S, L * B2 * H * S)
        in_maps.append({"wall": pack_wall(m4c, bptc),
                        "bpp": bpp.astype(np.float32)})

    return in_maps


def _erf(x):
    # vectorized erf via numpy (no scipy dependency)
    from math import erf
    return np.vectorize(erf)(x)


def _gelu_exact(x):
    x64 = x.astype(np.float64)
    return (x64 * 0.5 * (1.0 + _erf(x64 / math.sqrt(2.0)))).astype(np.float64)


def host_readout(xfinal, inp):
    """xfinal: [B, S, D] fp32 (pre-final-LN). Returns logits [B]."""
    f64 = np.float64
    x = xfinal.astype(f64)
    lnf_s = np.asarray(inp["lnf_s"], f64)
    lnf_b = np.asarray(inp["lnf_b"], f64)
    m = x.mean(-1, keepdims=True)
    v = ((x - m) ** 2).mean(-1, keepdims=True)
    xn = (x - m) / np.sqrt(v + 1e-5) * lnf_s + lnf_b

    P = np.asarray(inp["P"], f64)
    pad = np.broadcast_to(P, (xn.shape[0], GRID * GRID - S, D))
    grid = np.concatenate([xn, pad], 1).reshape(-1, GRID, GRID, D)
    grid = grid.transpose(0, 3, 1, 2)                   # [B, d, 12, 12]

    sc_w = np.asarray(inp["sc_w"], f64)                 # [d, d, 2, 2]
    sc_b = np.asarray(inp["sc_b"], f64)
    Bn = grid.shape[0]
    K = GRID // 2
    # strided 2x2 conv
    g = grid.reshape(Bn, D, K, 2, K, 2)
    xconv = np.einsum("bchpwq,ocpq->bohw", g, sc_w) + sc_b[None, :, None, None]
    xconv = _gelu_exact(xconv)

    dr_w = np.asarray(inp["dr_w"], f64)
    dr_b = np.asarray(inp["dr_b"], f64)
    xdr = np.einsum("bdhw,rd->brhw", xconv, dr_w) + dr_b[None, :, None, None]
    xdr = _gelu_exact(xdr)
    xp = xdr.mean(axis=2)                               # [B, rd, K]
    xp = xp.transpose(0, 2, 1).reshape(Bn * K, -1)      # [B*K, rd]

    rb1_w = np.asarray(inp["rb1_w"], f64)
    rb1_b = np.asarray(inp["rb1_b"], f64)
    rb2_w = np.asarray(inp["rb2_w"], f64)
    rb2_b = np.asarray(inp["rb2_b"], f64)
    for r in range(rb1_w.shape[0]):
        xp = xp + _gelu_exact(xp @ rb1_w[r] + rb1_b[r]) @ rb2_w[r] + rb2_b[r]
    out_w = np.asarray(inp["out_w"], f64)
    out_b = np.asarray(inp["out_b"], f64)
    logits = (xp @ out_w + out_b).reshape(Bn, K).mean(axis=1)
    return logits.astype(np.float32)


# --------------------------------------------------------------------------
# entry point
# --------------------------------------------------------------------------

def _get_graph():
    if "nc" not in _CACHE:
        _CACHE["nc"] = build_graph()
    return _CACHE["nc"]


def kernel(**inputs):
    nc = _get_graph()
    in_maps = prepare_inputs(inputs)
    core_ids = list(range(NCORES))
    res = run_bass_kernel_spmd(nc, in_maps, core_ids,
                               trace=bool(os.environ.get("KTRACE")))
    _CACHE["last_result"] = res
    # gather: results[i]['xout'] is [D, N] with token order (b, s)
    xf = np.zeros((B, S, D), np.float32)
    for c in range(NCORES):
        xo = np.asarray(res.results[c]["xout"], np.float32)  # [D, 240]
        xf[c * B2:(c + 1) * B2] = xo.reshape(D, B2, S).transpose(1, 2, 0)
    return host_readout(xf, inputs)


# revision 31
# speedup vs baseline: 1.1746x; 1.1746x over previous
"""AlphaQubit-like recurrent transformer on 8 TRN2 NeuronCores.

Strategy:
- Data-parallel over batch: B=16 -> 2 per core, params replicated, no
  collectives. Host shards inputs / concatenates outputs.
- Feature-major on-device layout: activations [d=128 partitions, tokens free].
- The two batches per core run as two independent dependency chains
  ([d, 120] tiles) so their ops interleave across engines.
- Embedding chunks (one per cycle t) are emitted inside the main loop two
  iterations ahead; the Tile scheduler uses them to fill engine gaps.
- bf16 matmul operands, fp32 PSUM accumulation.
- Single ACT table set (natural_log_exp_and_others): LayerNorm rstd via
  exp(-0.5*ln(var+eps)), softmax via exp, gelu via exp-form tanh approx.
- LN scale/bias, attention scale, and small biases folded into weights on host.
- Readout tail (post-recurrence LN + conv head + residual MLP, <1% of FLOPs)
  computed on host in fp32.
"""

import math
import os
import sys

import numpy as np

sys.path.insert(0, "/opt/trn_rl_repo")

import concourse.bass as bass
import concourse.bacc as bacc
import concourse.tile as tile
from concourse.tile import add_dep_helper
from concourse import mybir
from concourse.bass_utils import run_bass_kernel_spmd

import ml_dtypes

BF16 = ml_dtypes.bfloat16

# model dims
B, T, S, D = 16, 8, 120, 128
L, H, DA, DM, DB = 2, 4, 32, 32, 32
NCORES = 8
B2 = B // NCORES          # 2 batches per core
N = B2 * S                # 240 tokens (both batches)
NE = T * B2 * S           # 1920 tokens in embed phase
GRID = 12
RD, NRB = 48, 16

# gelu (tanh approx) constants, computed via exp:
#   gelu(x) ~= x * sigmoid(2u), u = sqrt(2/pi) * (x + r*x^3)
#   e = exp(-2u) = exp(sg * r * (x^2 + 1/r) * x)
R_G = 0.044715
SG = -2.0 * math.sqrt(2.0 / math.pi)
EXP_SCALE = SG * R_G     # ACT scale for exp input (applied to (x^2+1/r)*x)
INV_RG = 1.0 / R_G

F32 = mybir.dt.float32
BF = mybir.dt.bfloat16
AF = mybir.ActivationFunctionType
ALU = mybir.AluOpType

_CACHE = {}

# wall segment layout (shared by device + host): first segments feed the
# embed phase so its compute can start while the rest still streams in.
WALL_SEGS = [
    ("m4", 4, NE), ("ce", D, NE), ("w4", 4, D), ("wer", D, 2 * 2 * D),
    ("ident", S, S), ("bpt", S, L * B2 * H * S),
    ("wqk", D, L * 2 * D), ("wv", D, L * D), ("wo", D, L * D),
    ("wf1", D, L * 4 * D), ("wf2", D, L * 2 * D), ("wcv", D, L * 3 * D),
    ("hsel", S, 16), ("ind4", 4, D),
]
WALL_COLS = sum(c for _, _, c in WALL_SEGS)
EARLY_COLS = 2 * NE + D + 4 * D  # m4, ce, w4, wer


# --------------------------------------------------------------------------
# device graph
# --------------------------------------------------------------------------

def _patched_act_tables(arch):
    # The stock picker maps Ln->natural_log and Exp->exp_and_others,
    # reloading the ACT table (~1.3us) on every switch. Empty those two
    # sets so both functions resolve to natural_log_exp_and_others
    # (positional set ids must stay intact).
    from concourse.hw_specs import get_activation_tables as real
    tabs = dict(real(arch))
    out = {}
    for k, v in tabs.items():
        if k in ("natural_log", "exp_and_others", "exp_and_friends"):
            out[k] = set()
        else:
            out[k] = v
    return out


def build_graph():
    bacc_mod = sys.modules["concourse.bacc"]
    bacc_mod.get_activation_tables = _patched_act_tables
    nc = bacc.Bacc(None)

    wall = nc.declare_dram_parameter("wall", [D, WALL_COLS], BF, isOutput=False)
    bpp = nc.declare_dram_parameter("bpp", [D, 22], F32, isOutput=False)
    xout = nc.declare_dram_parameter("xout", [D, N], F32, isOutput=True)

    # per-partition bias column indices in bpp
    BO = lambda l: 4 + l            # 4,5
    BF2 = lambda l: 6 + l           # 6,7
    BER2 = lambda r: 8 + r          # 8,9
    BF1 = lambda l, s: 10 + l * 4 + s   # 10..17  (s in 0..3: a0,a1,g0,g1)
    BCV = lambda l: 18 + l          # 18,19
    BER1 = lambda r: 20 + r         # 20,21

    with tile.TileContext(nc) as tc:
        singles = tc.alloc_tile_pool(name="singles", bufs=1)
        work = tc.alloc_tile_pool(name="work", bufs=4)
        xpool = tc.alloc_tile_pool(name="xpool", bufs=4)
        ps = tc.alloc_tile_pool(name="ps", bufs=2, space="PSUM")

        # ---- load constants/weights into SBUF (embed segs first) ----
        s_wall = singles.tile([D, WALL_COLS], BF, tag="wall")
        nc.sync.dma_start(out=s_wall[:, 0:EARLY_COLS], in_=wall[:, 0:EARLY_COLS])
        nc.sync.dma_start(out=s_wall[:, EARLY_COLS:], in_=wall[:, EARLY_COLS:])
        s_bpp = singles.tile([D, 22], F32, tag="bpp")
        nc.sync.dma_start(out=s_bpp, in_=bpp[:, :])

        seg_off = {}
        off = 0
        for nm, rows, cols in WALL_SEGS:
            seg_off[nm] = off
            off += cols

        def seg(nm, rows, cols):
            o = seg_off[nm]
            return s_wall[0:rows, o:o + cols]

        s_m4 = seg("m4", 4, NE)
        s_ce = seg("ce", D, NE)
        s_bpt = seg("bpt", S, L * B2 * H * S)
        s_wqk = seg("wqk", D, L * 2 * D)
        s_wv = seg("wv", D, L * D)
        s_wo = seg("wo", D, L * D)
        s_wf1 = seg("wf1", D, L * 4 * D)
        s_wf2 = seg("wf2", D, L * 2 * D)
        s_wcv = seg("wcv", D, L * 3 * D)
        s_wer = seg("wer", D, 2 * 2 * D)
        s_w4 = seg("w4", 4, D)
        s_id = seg("ident", S, S)

        onesc = singles.tile([D, D], BF)       # 1/128 (stat matmuls)
        nc.vector.memset(onesc, 1.0 / 128.0)
        onescf = singles.tile([D, D], F32)     # 1/128 fp32 (mean of fp32 X)
        nc.vector.memset(onescf, 1.0 / 128.0)
        hsel = seg("hsel", S, 16)
        ind4 = seg("ind4", 4, D)
        ones1 = singles.tile([D, D], BF)       # 1.0 (denominator/broadcast)
        nc.vector.memset(ones1, 1.0)
        onesf = singles.tile([1, 32], F32)     # 1.0 fp32 (recip broadcast)
        nc.vector.memset(onesf, 1.0)
        eps_t = singles.tile([1, 1], F32)
        nc.vector.memset(eps_t, 1e-5)
        zero_t = singles.tile([D, 1], F32)
        nc.vector.memset(zero_t, 0.0)

        bias_ap = lambda c: s_bpp[:, c:c + 1]

        # ---- helper: layernorm (feature-major) -> xn bf16 [D, n] ----
        # psum arena `slot` [D, 480] (one bank): mean bcast 0:n, var row
        # n:2n, rstd bcast 2n:3n (or a second slot when n > 160).
        def layer_norm(x, n, arena_tag, tp, abufs=2):
            slot = ps.tile([D, 480], F32, tag=arena_tag, bufs=abufs)
            mb = slot[:, 0:n]
            nc.tensor.matmul(mb, onescf, x, start=True, stop=True)
            xc = work.tile([D, n], BF, tag=tp + "ln_xc", bufs=4)
            nc.vector.tensor_sub(xc, x, mb)
            sq = work.tile([D, n], BF, tag=tp + "ln_sq", bufs=4)
            nc.scalar.activation(sq, xc, AF.Square, bias=zero_t, scale=1.0)
            vr = slot[0:1, n:2 * n]
            nc.tensor.matmul(vr, onesc[:, 0:1], sq, start=True, stop=True)
            lnr = work.tile([1, n], F32, tag=tp + "ln_lnr", bufs=4)
            nc.scalar.activation(lnr, vr, AF.Ln, bias=eps_t[0:1, :], scale=1.0)
            rsr = work.tile([1, n], BF, tag=tp + "ln_rsr", bufs=4)
            nc.scalar.activation(rsr, lnr, AF.Exp, bias=zero_t[0:1, :], scale=-0.5)
            if 3 * n <= 480:
                rb = slot[:, 2 * n:3 * n]
            else:
                rb = ps.tile([D, 480], F32, tag=arena_tag, bufs=abufs, name="ln_rbs")[:, 0:n]
            nc.tensor.matmul(rb, ones1[0:1, 0:D], rsr, start=True, stop=True)
            xn = work.tile([D, n], BF, tag=tp + "ln_xn", bufs=4)
            nc.vector.tensor_mul(xn, xc, rb)
            return xn

        # ---- helper: gelu(a) -> gl bf16 [D, n]; a is sbuf bf16 ----
        def gelu(a, n, tag, w_eng=None):
            x2 = work.tile([D, n], BF, tag=tag + "_x2")
            nc.scalar.activation(x2, a, AF.Square, bias=zero_t, scale=1.0)
            w = work.tile([D, n], BF, tag=tag + "_w")
            (w_eng or nc.vector).scalar_tensor_tensor(
                w, x2, INV_RG, a, op0=ALU.add, op1=ALU.mult)
            e = work.tile([D, n], F32, tag=tag + "_e")
            nc.scalar.activation(e, w, AF.Exp, bias=zero_t, scale=EXP_SCALE)
            dd = work.tile([D, n], F32, tag=tag + "_dd")
            nc.vector.tensor_scalar_add(dd, e, 1.0)
            rc = work.tile([D, n], F32, tag=tag + "_rc")
            nc.vector.reciprocal_approx_fast(out=rc, in_=dd)
            gl = work.tile([D, n], BF, tag=tag + "_gl")
            nc.gpsimd.tensor_mul(gl, rc, a)
            return gl

        # ================= embed chunk (one cycle t, both batches) =======
        e_tiles = [None] * T

        def emit_embed_chunk(t):
            sl = slice(t * N, (t + 1) * N)
            h0p = ps.tile([D, 480], F32, tag="E", bufs=2, name="e_h0p")[:, 0:N]
            nc.tensor.matmul(h0p, s_w4, s_m4[:, sl], start=True, stop=True)
            h = work.tile([D, N], F32, tag="emb_h", bufs=3)
            nc.vector.tensor_add(h, h0p, s_ce[:, sl])
            for r in range(2):
                xn = layer_norm(h, N, "E", "e_")
                f1p = ps.tile([D, 480], F32, tag="E", bufs=2, name="e_f1p")[:, 0:N]
                nc.tensor.matmul(f1p, s_wer[:, (r * 2) * D:(r * 2) * D + D], xn,
                                 start=True, stop=True)
                a = work.tile([D, N], BF, tag="emb_a")
                nc.scalar.activation(a, f1p, AF.Identity,
                                     bias=bias_ap(BER1(r)), scale=1.0)
                gl = gelu(a, N, "emb_g")
                f2p = ps.tile([D, 480], F32, tag="E", bufs=2, name="e_f2p")[:, 0:N]
                nc.tensor.matmul(f2p, s_wer[:, (r * 2 + 1) * D:(r * 2 + 1) * D + D],
                                 gl, start=True, stop=True)
                if r == 0:
                    hn = work.tile([D, N], F32, tag="emb_h", bufs=3)
                else:
                    hn = xpool.tile([D, N], F32, tag=f"e{t}", bufs=1)
                nc.vector.scalar_tensor_tensor(
                    hn, f2p, bias_ap(BER2(r)), h, op0=ALU.add, op1=ALU.add)
                h = hn
            e_tiles[t] = h  # fp32 [D, N], pre-scaled by 1/sqrt(2)

        # ================= main loop stages (one batch chain each) =======
        NB = S  # 120 tokens per batch chain

        def attn(X, b, l):
            xn = layer_norm(X, NB, "A", "m_")
            # B-arena bank 1: Q [0:32, (h,i)], K [32:64, (h,i)]
            qkp = ps.tile([D, 480], F32, tag="B", bufs=2, name="qkp")
            for hh in range(H):
                nc.tensor.matmul(
                    qkp[0:DA, hh * NB:(hh + 1) * NB],
                    s_wqk[:, (l * 2) * D + hh * DA:(l * 2) * D + (hh + 1) * DA],
                    xn, start=True, stop=True)
                nc.tensor.matmul(
                    qkp[DA:2 * DA, hh * NB:(hh + 1) * NB],
                    s_wqk[:, (l * 2 + 1) * D + hh * DA:(l * 2 + 1) * D + (hh + 1) * DA],
                    xn, start=True, stop=True,
                    tile_position=(0, DA), skip_group_check=True)
            qst = work.tile([DA, H * NB], BF, tag="qst")
            nc.vector.tensor_copy(qst, qkp[0:DA, :])
            kst = work.tile([DA, H * NB], BF, tag="kst")
            nc.vector.tensor_copy(kst, qkp[DA:2 * DA, :])
            # B-arena bank 2: V [0:120, 0:128], bc 128:248, ot 248:368, zt 368:488
            varena = ps.tile([D, 512], F32, tag="B", bufs=2, name="varena")
            vp = varena[0:NB, 0:D]
            nc.tensor.matmul(vp, xn, s_wv[:, l * D:(l + 1) * D],
                             start=True, stop=True)
            vsb = work.tile([NB, D], BF, tag="vsb")
            nc.vector.tensor_copy(vsb, vp)

            # scores in two head-pair groups so exp(h01) overlaps scores(h23)
            boff = l * B2 * H * S + b * H * S
            ex = work.tile([S, H * S], BF, tag="ex")
            for g in range(2):
                scg = ps.tile([D, 480], F32, tag="C", bufs=2, name="scg")[0:S, 0:2 * S]
                go = 2 * g * S
                nc.tensor.matmul(scg, s_id, s_bpt[:, boff + go:boff + go + 2 * S],
                                 start=True, stop=False)
                for hh in range(2 * g, 2 * g + 2):
                    nc.tensor.matmul(
                        scg[:, (hh - 2 * g) * S:(hh - 2 * g + 1) * S],
                        kst[:, hh * NB:(hh + 1) * NB], qst[:, hh * NB:(hh + 1) * NB],
                        start=False, stop=(hh == 2 * g + 1))
                nc.scalar.activation(ex[:, go:go + 2 * S], scg, AF.Exp,
                                     bias=zero_t[0:S, :], scale=1.0)

            # denominators into [4(h), 120]: row h from head-h block of ex
            dn = ps.tile([D, 480], F32, tag="C", bufs=2, name="dn")[0:4, 0:NB]
            for hh in range(H):
                nc.tensor.matmul(dn, hsel[:, 4 * hh:4 * hh + 4],
                                 ex[:, hh * S:(hh + 1) * S],
                                 start=(hh == 0), stop=(hh == H - 1))
            rr = work.tile([4, NB], F32, tag="rr")
            nc.vector.reciprocal_approx_fast(out=rr, in_=dn)
            rrb = work.tile([4, NB], BF, tag="rrb")
            nc.vector.tensor_copy(rrb, rr)
            nc.tensor.matmul(varena[:, D:D + NB], ind4, rrb,
                             start=True, stop=True)
            for hh in range(H):
                nc.tensor.matmul(varena[hh * DM:(hh + 1) * DM, D + NB:D + 2 * NB],
                                 vsb[:, hh * DM:(hh + 1) * DM],
                                 ex[:, hh * S:(hh + 1) * S],
                                 start=True, stop=True,
                                 tile_position=(0, hh * DM), skip_group_check=True)
            bcs = work.tile([D, NB], BF, tag="bcs")
            nc.scalar.activation(bcs, varena[:, D:D + NB], AF.Copy)
            on = work.tile([D, NB], BF, tag="on")
            nc.vector.tensor_mul(on, varena[:, D + NB:D + 2 * NB], bcs)
            zt = varena[:, D + 2 * NB:D + 3 * NB]
            nc.tensor.matmul(zt, s_wo[:, l * D:(l + 1) * D], on,
                             start=True, stop=True)
            x2t = xpool.tile([D, NB], F32, tag=f"x{b}", bufs=3, name="x2t")
            nc.vector.scalar_tensor_tensor(
                x2t, zt, bias_ap(BO(l)), X, op0=ALU.add, op1=ALU.add)
            return x2t

        def ffn(X, b, l):
            xn2 = layer_norm(X, NB, "A", "m_")
            f1 = ps.tile([D, 480], F32, tag="C", bufs=2, name="f1")
            for s4 in range(4):
                nc.tensor.matmul(
                    f1[:, s4 * NB:(s4 + 1) * NB],
                    s_wf1[:, l * 4 * D + s4 * D: l * 4 * D + (s4 + 1) * D],
                    xn2, start=True, stop=True)
            a_s = work.tile([D, 2 * NB], BF, tag="ffa")
            nc.scalar.activation(a_s, f1[:, 0:2 * NB], AF.Identity,
                                 bias=bias_ap(BF1(l, 0)), scale=1.0)
            gl = gelu(a_s, 2 * NB, "ffg")
            ffo = work.tile([D, 2 * NB], BF, tag="ffo")
            nc.vector.tensor_mul(ffo, f1[:, 2 * NB:4 * NB], gl)
            zf = ps.tile([D, 480], F32, tag="B", bufs=2, name="zf")[:, 0:NB]
            for s2 in range(2):
                nc.tensor.matmul(zf, s_wf2[:, (l * 2 + s2) * D:(l * 2 + s2 + 1) * D],
                                 ffo[:, s2 * NB:(s2 + 1) * NB],
                                 start=(s2 == 0), stop=(s2 == 1))
            x3t = xpool.tile([D, NB], F32, tag=f"x{b}", bufs=3, name="x3t")
            nc.vector.scalar_tensor_tensor(
                x3t, zf, bias_ap(BF2(l)), X, op0=ALU.add, op1=ALU.add)
            return x3t

        def conv(X, b, l):
            x3b = work.tile([D, NB], BF, tag="x3b")
            nc.vector.tensor_copy(x3b, X)
            cv = ps.tile([D, 480], F32, tag="B", bufs=2, name="cv")[:, 0:NB]
            k0 = l * 3 * D
            nc.tensor.matmul(cv, s_wcv[:, k0 + D:k0 + 2 * D], x3b,
                             start=True, stop=False)
            nc.tensor.matmul(cv[:, 1:NB], s_wcv[:, k0:k0 + D],
                             x3b[:, 0:NB - 1], start=False, stop=False)
            nc.tensor.matmul(cv[:, 0:NB - 1], s_wcv[:, k0 + 2 * D:k0 + 3 * D],
                             x3b[:, 1:NB], start=False, stop=True)
            acv = work.tile([D, NB], BF, tag="acv")
            nc.scalar.activation(acv, cv, AF.Identity,
                                 bias=bias_ap(BCV(l)), scale=1.0)
            gl = gelu(acv, NB, "cvg")
            x4t = xpool.tile([D, NB], F32, tag=f"x{b}", bufs=3, name="x4t")
            nc.vector.tensor_add(x4t, gl, X)
            return x4t

        # ================= schedule =================
        emit_embed_chunk(0)
        emit_embed_chunk(1)

        Xb = [None, None]
        for t in range(T):
            e_t = e_tiles[t]
            for b in range(B2):
                esl = e_t[:, b * NB:(b + 1) * NB]
                if t == 0:
                    Xb[b] = esl  # X0 = (0 + E0)/sqrt(2), scaling pre-folded
                else:
                    xnew = xpool.tile([D, NB], F32, tag=f"x{b}", bufs=3)
                    nc.vector.scalar_tensor_tensor(
                        xnew, Xb[b], 1.0 / math.sqrt(2.0), esl,
                        op0=ALU.mult, op1=ALU.add)
                    Xb[b] = xnew
            STAGE = os.environ.get("K_STAGE", "full")
            for l in range(L):
                if STAGE in ("full", "attn", "ffn"):
                    for b in range(B2):
                        Xb[b] = attn(Xb[b], b, l)
                if STAGE in ("full", "ffn"):
                    for b in range(B2):
                        Xb[b] = ffn(Xb[b], b, l)
                if STAGE in ("full",):
                    for b in range(B2):
                        Xb[b] = conv(Xb[b], b, l)
            if t + 2 < T:
                emit_embed_chunk(t + 2)

        # write out final X (f32)
        for b in range(B2):
            nc.sync.dma_start(out=xout[:, b * NB:(b + 1) * NB], in_=Xb[b])

        for p in (ps, xpool, work, singles):
            p.release()

    nc.compile()
    return nc


# --------------------------------------------------------------------------
# host pre/post-processing
# --------------------------------------------------------------------------

def _bf(x):
    return np.asarray(x, dtype=np.float32).astype(BF16)


def prepare_inputs(inp):
    """Build per-core input maps (numpy) from full fp32 inputs."""
    f = {k: np.asarray(v, dtype=np.float32) for k, v in inp.items()
         if k not in ("stab_ids", "cycle_ids")}
    stab_ids = np.asarray(inp["stab_ids"])
    cycle_ids = np.asarray(inp["cycle_ids"])

    scale = 1.0 / math.sqrt(DA)
    isq2 = 1.0 / math.sqrt(2.0)

    # ---- replicated weights ----
    # wqk: ln1-folded, q side also attn-scaled
    wqk = np.zeros((D, L * 2 * D), np.float32)
    bqk = np.zeros((D, 4), np.float32)
    for l in range(L):
        wq = f["Wq"][l].transpose(1, 0, 2).reshape(D, H * DA)   # [d, (h,e)]
        wk = f["Wk"][l].transpose(1, 0, 2).reshape(D, H * DA)
        wq_f = f["ln1_s"][l][:, None] * wq
        wk_f = f["ln1_s"][l][:, None] * wk
        bq_f = (f["bq"][l].reshape(-1) + f["ln1_b"][l] @ wq) * scale
        bk_f = f["bk"][l].reshape(-1) + f["ln1_b"][l] @ wk
        wqk[:, (l * 2) * D:(l * 2) * D + D] = wq_f * scale
        wqk[:, (l * 2 + 1) * D:(l * 2 + 1) * D + D] = wk_f
        bqk[:, l * 2 + 0] = bq_f
        bqk[:, l * 2 + 1] = bk_f

    wv = np.zeros((D, L * D), np.float32)
    wo = np.zeros((D, L * D), np.float32)
    bo_all = np.zeros((D, L), np.float32)
    for l in range(L):
        wv_r = f["Wv"][l].transpose(1, 0, 2).reshape(D, H * DM)
        wv_f = f["ln1_s"][l][:, None] * wv_r
        bv_f = f["bv"][l].reshape(-1) + f["ln1_b"][l] @ wv_r
        wv[:, l * D:(l + 1) * D] = wv_f
        wo[:, l * D:(l + 1) * D] = f["Wo"][l]         # [hm, d]
        bo_all[:, l] = f["bo"][l] + bv_f @ f["Wo"][l]

    wf1 = np.zeros((D, L * 4 * D), np.float32)
    bf1 = np.zeros((D, 8), np.float32)
    for l in range(L):
        w = f["ln2_s"][l][:, None] * f["f1_w"][l]      # [d, 512]
        bias = f["f1_b"][l] + f["ln2_b"][l] @ f["f1_w"][l]
        wf1[:, l * 4 * D:(l + 1) * 4 * D] = w
        for s4 in range(4):
            bf1[:, l * 4 + s4] = bias[s4 * D:(s4 + 1) * D]

    wf2 = np.zeros((D, L * 2 * D), np.float32)
    bf2 = np.zeros((D, L), np.float32)
    for l in range(L):
        for s2 in range(2):
            wf2[:, (l * 2 + s2) * D:(l * 2 + s2 + 1) * D] = \
                f["f2_w"][l][s2 * D:(s2 + 1) * D]
        bf2[:, l] = f["f2_b"][l]

    wcv = np.zeros((D, L * 3 * D), np.float32)
    bcv = np.zeros((D, L), np.float32)
    for l in range(L):
        for k in range(3):
            wcv[:, (l * 3 + k) * D:(l * 3 + k + 1) * D] = f["conv_w"][l][:, :, k].T
        bcv[:, l] = f["conv_b"][l]

    wer = np.zeros((D, 4 * D), np.float32)
    ber1 = np.zeros((D, 2), np.float32)
    ber2 = np.zeros((D, 2), np.float32)
    for r in range(2):
        w1 = f["er_ln_s"][r][:, None] * f["er_fc1_w"][r]
        b1 = f["er_fc1_b"][r] + f["er_ln_b"][r] @ f["er_fc1_w"][r]
        wer[:, (r * 2) * D:(r * 2) * D + D] = w1
        wer[:, (r * 2 + 1) * D:(r * 2 + 1) * D + D] = f["er_fc2_w"][r] * isq2
        ber1[:, r] = b1
        ber2[:, r] = f["er_fc2_b"][r] * isq2

    w4 = np.stack([f["pm_w"], f["pe_w"], f["pl_w"], f["pel_w"]], 0)  # [4,d]

    assert np.abs(bqk).max() == 0.0, "qk biases must be zero (folded path)"
    assert np.abs(bf1).max() == 0.0, "f1 biases must be zero (fused a_s/ffo path)"
    bpp = np.zeros((D, 22), np.float32)
    bpp[:, 0:4] = bqk
    bpp[:, 4:6] = bo_all
    bpp[:, 6:8] = bf2
    bpp[:, 8:10] = ber2
    bpp[:, 10:18] = bf1
    bpp[:, 18:20] = bcv
    bpp[:, 20:22] = ber1

    # const embedding [d, (t, s)] replicated over b, scaled by 1/sqrt(2)
    pos = f["stab_emb"][stab_ids]                      # [S, d]
    cyc = f["cyc_emb"][cycle_ids]                      # [T, d]
    cbias = f["pm_b"] + f["pe_b"] + f["pl_b"] + f["pel_b"]
    const_ts = (cbias[None, None, :] + pos[None, :, :] + cyc[:, None, :]) * isq2
    # [T, S, d] -> [d, (t,b,s)]
    ce_full = np.repeat(const_ts[:, None, :, :], B2, axis=1)  # [T,B2,S,d]
    ce = ce_full.transpose(3, 0, 1, 2).reshape(D, NE)

    ident = np.eye(S, dtype=np.float32)

    def pack_wall(m4c, bptc):
        hsel = np.zeros((S, 16), np.float32)
        for h in range(H):
            hsel[:, 5 * h] = 1.0
        ind4 = np.zeros((4, D), np.float32)
        for g in range(4):
            ind4[g, 32 * g:32 * (g + 1)] = 1.0
        arrs = {"m4": m4c, "ce": ce, "w4": w4, "wer": wer, "ident": ident,
                "bpt": bptc, "wqk": wqk, "wv": wv, "wo": wo,
                "wf1": wf1, "wf2": wf2, "wcv": wcv,
                "hsel": hsel, "ind4": ind4}
        wallm = np.zeros((D, WALL_COLS), np.float32)
        o = 0
        for nm, r, c in WALL_SEGS:
            wallm[0:r, o:o + c] = arrs[nm]
            o += c
        return _bf(wallm)

    # ---- per-core sharded inputs ----
    # Bp^T: [l, b, h, j(k), i(q)] scaled by 1/sqrt(da)
    bias_in = f["bias"]                                # [B, S, S, DB]
    Wb = f["Wb"]                                       # [L, DB, H]
    bp = np.einsum("bijd,ldh->lbhji", bias_in, Wb) * scale  # [L,B,H,S(j),S(i)]

    in_maps = []
    for c in range(NCORES):
        bsl = slice(c * B2, (c + 1) * B2)
        m4c = np.stack([f["meas"][bsl], f["event"][bsl], f["leak"][bsl],
                        f["event_leak"][bsl]], 0)       # [4, B2, T, S]
        m4c = (m4c.transpose(0, 2, 1, 3).reshape(4, NE)) * isq2  # (t,b,s)
        bptc = bp[:, bsl]                               # [L, B2, H, S, S]
        bptc = bptc.transpose(3, 0, 1, 2, 4).reshape(S, L * B2 * H * S)
        in_maps.append({"wall": pack_wall(m4c, bptc),
                        "bpp": bpp.astype(np.float32)})

    return in_maps


def _erf(x):
    # vectorized erf via numpy (no scipy dependency)
    from math import erf
    return np.vectorize(erf)(x)


def _gelu_exact(x):
    x64 = x.astype(np.float64)
    return (x64 * 0.5 * (1.0 + _erf(x64 / math.sqrt(2.0)))).astype(np.float64)


def host_readout(xfinal, inp):
    """xfinal: [B, S, D] fp32 (pre-final-LN). Returns logits [B]."""
    f64 = np.float64
    x = xfinal.astype(f64)
    lnf_s = np.asarray(inp["lnf_s"], f64)
    lnf_b = np.asarray(inp["lnf_b"], f64)
    m = x.mean(-1, keepdims=True)
    v = ((x - m) ** 2).mean(-1, keepdims=True)
    xn = (x - m) / np.sqrt(v + 1e-5) * lnf_s + lnf_b

    P = np.asarray(inp["P"], f64)
    pad = np.broadcast_to(P, (xn.shape[0], GRID * GRID - S, D))
    grid = np.concatenate([xn, pad], 1).reshape(-1, GRID, GRID, D)
    grid = grid.transpose(0, 3, 1, 2)                   # [B, d, 12, 12]

    sc_w = np.asarray(inp["sc_w"], f64)                 # [d, d, 2, 2]
    sc_b = np.asarray(inp["sc_b"], f64)
    Bn = grid.shape[0]
    K = GRID // 2
    # strided 2x2 conv
    g = grid.reshape(Bn, D, K, 2, K, 2)
    xconv = np.einsum("bchpwq,ocpq->bohw", g, sc_w) + sc_b[None, :, None, None]
    xconv = _gelu_exact(xconv)

    dr_w = np.asarray(inp["dr_w"], f64)
    dr_b = np.asarray(inp["dr_b"], f64)
    xdr = np.einsum("bdhw,rd->brhw", xconv, dr_w) + dr_b[None, :, None, None]
    xdr = _gelu_exact(xdr)
    xp = xdr.mean(axis=2)                               # [B, rd, K]
    xp = xp.transpose(0, 2, 1).reshape(Bn * K, -1)      # [B*K, rd]

    rb1_w = np.asarray(inp["rb1_w"], f64)
    rb1_b = np.asarray(inp["rb1_b"], f64)
    rb2_w = np.asarray(inp["rb2_w"], f64)
    rb2_b = np.asarray(inp["rb2_b"], f64)
    for r in range(rb1_w.shape[0]):
        xp = xp + _gelu_exact(xp @ rb1_w[r] + rb1_b[r]) @ rb2_w[r] + rb2_b[r]
    out_w = np.asarray(inp["out_w"], f64)
    out_b = np.asarray(inp["out_b"], f64)
    logits = (xp @ out_w + out_b).reshape(Bn, K).mean(axis=1)
    return logits.astype(np.float32)


# --------------------------------------------------------------------------
# entry point
# --------------------------------------------------------------------------

def _get_graph():
    if "nc" not in _CACHE:
        _CACHE["nc"] = build_graph()
    return _CACHE["nc"]


def kernel(**inputs):
    nc = _get_graph()
    in_maps = prepare_inputs(inputs)
    core_ids = list(range(NCORES))
    res = run_bass_kernel_spmd(nc, in_maps, core_ids,
                               trace=bool(os.environ.get("KTRACE")))
    _CACHE["last_result"] = res
    # gather: results[i]['xout'] is [D, N] with token order (b, s)
    xf = np.zeros((B, S, D), np.float32)
    for c in range(NCORES):
        xo = np.asarray(res.results[c]["xout"], np.float32)  # [D, 240]
        xf[c * B2:(c + 1) * B2] = xo.reshape(D, B2, S).transpose(1, 2, 0)
    return host_readout(xf, inputs)
